# revision 24
# baseline (speedup 1.0000x reference)
"""Multi-head causal attention (b=1, s=4096, d=1024, 16 heads) on 8 NeuronCores.

Sharding: tensor-parallel over heads — 2 heads per core. Each core computes
Q/K/V projections for its heads, causal attention, and its row-slice of the
output projection (partial sum). Host sums the 8 partial outputs.

Device layout notes:
 - x is pre-transposed + fp8(e4m3)-cast on host: xT [1024, 4096]. Wq/Wk/Wv are
   scaled by 32 and fp8-cast so the projection matmuls can run in fp8
   DoubleRow perf mode (2 contraction tiles per instruction, 2x throughput).
   The 32x weight scale is compensated in the exp scale (Q,K) and in the
   softmax denominator (V: the ones column of vaug holds 32.0).
 - Scores are computed transposed (S^T [k, q]) so the probs tile is the PV
   matmul's *stationary* operand (full 128-wide): PV streams V (65 cols incl.
   the denominator column) per 128x128 probs block, halving PV's PE time vs
   streaming queries. PV output O is [q, d]; it is normalized with a
   per-partition scalar and transposed back to O^T [d, q] on the PE (cheap
   identity-matmul transposes) for the output projection.
 - exp has no max-subtraction: scores ~ N(0,1) by construction, fp32 PSUM
   holds exp easily. A fraction of the exps runs as a fast-exp bit trick
   (u16 = round(a*s + b) reinterpreted as bf16) on DVE/Pool to unload the
   Activation engine, which is otherwise the bottleneck.
 - Causal masking is multiplicative on exp(S^T) using a precomputed
   [128, 1024] sliding mask (only diagonal blocks need it).
"""

import numpy as np
import ml_dtypes

import concourse.bass as bass
import concourse.mybir as mybir
import concourse.tile as tile
from concourse import bacc
from concourse.bass_utils import run_bass_kernel_spmd

BF16 = ml_dtypes.bfloat16
S = 4096          # sequence length
D = 1024          # model dim
NCORES = 8
HL = 2            # heads per core
HD = 64           # head dim
DK = D // 128     # 8 contraction tiles for projections
NQC = S // 512    # 8 query chunks of 512
NKT = S // 128    # 32 key tiles of 128
FP32 = mybir.dt.float32
BF = mybir.dt.bfloat16
FP8 = mybir.dt.float8e4
U16 = mybir.dt.uint16
EXP = mybir.ActivationFunctionType.Exp
MULT = mybir.AluOpType.mult
ADD = mybir.AluOpType.add
DR = mybir.MatmulPerfMode.DoubleRow

WSCALE = 32.0       # fp8 V-weight pre-scale (host); Q/K stay bf16 unscaled
SSCALE = 0.125      # exp scale: 1/sqrt(64)
# fast-exp: exp(s*SSCALE) ~ bf16_bits(round(A*s + B)); bf16 mantissa is
# 7 bits so the bit-trick scale is 2^7 = 128
FE_A = 128 * 1.4426950408889634 * SSCALE
FE_B = 128 * (127 - 0.043) + 0.3
# each full exp tile is split column-wise: ACT computes the first 3/4
# natively, DVE computes the rest via the fast-exp bit trick — cuts the
# latency of each exp (which gates the 2-slot scores-PSUM rotation) and
# balances the engines' totals. Diagonal tiles run entirely on Pool as a
# fused fast-exp+mask (out = (s*A) + Bmask; masked entries get Bmask=0 so
# their bf16 bit pattern is ~0).
def _exp_split(w):
    return (w * 3 // 4) // 128 * 128


# scheduling/placement knobs (tuned via TimelineSim sweep)
# NOTE: GPSIMD/Pool cannot access PSUM on trn2 — every PSUM-touching op
# must go to PE/ACT/DVE. Pool only gets SBUF->SBUF work (memsets).
CFG = {
    "act_num": 2, "act_den": 3,   # ACT share of full-tile exps
    "yact_num": 0, "yact_den": 8, # ACT share of y staging copies
    "lag": 5,
    "proj_ahead": 2,
    "no_fastexp": False,  # debug: all exps on ACT, multiplicative diag mask
    "no_fp8": True,       # fp8 V projection adds ~2% rel err for ~no speedup
}


def _build_program(repeat=1):
    nc = bacc.Bacc("TRN2", target_bir_lowering=False, debug=False, num_devices=NCORES)

    PDT = BF if CFG["no_fp8"] else FP8
    xT = nc.dram_tensor("xT", [D, S], BF, kind="ExternalInput").ap()
    xT8 = nc.dram_tensor("xT8", [D, S], PDT, kind="ExternalInput").ap()
    wq = nc.dram_tensor("wq", [D, 128], BF, kind="ExternalInput").ap()
    wk = nc.dram_tensor("wk", [D, 128], BF, kind="ExternalInput").ap()
    wv = nc.dram_tensor("wv", [D, 128], PDT, kind="ExternalInput").ap()
    wo = nc.dram_tensor("wo", [128, D], BF, kind="ExternalInput").ap()
    bmask = nc.dram_tensor("bmask", [128, 1024], FP32, kind="ExternalInput").ap()
    mask01 = nc.dram_tensor("mask01", [128, 1024], BF, kind="ExternalInput").ap()
    ident = nc.dram_tensor("ident", [128, 128], BF, kind="ExternalInput").ap()
    y = nc.dram_tensor("y", [S, D], BF, kind="ExternalOutput").ap()

    with tile.TileContext(nc) as tc:
        with (
            tc.tile_pool(name="persist", bufs=1) as pp,
            tc.tile_pool(name="stp", bufs=4, space="PSUM") as stp,
            tc.tile_pool(name="sdp", bufs=2, space="PSUM") as sdp,
            tc.tile_pool(name="pvp", bufs=2, space="PSUM") as pvp,
            tc.tile_pool(name="epool", bufs=14) as ep,
            tc.tile_pool(name="small", bufs=8) as sp,
            tc.tile_pool(name="onp", bufs=3) as onp,
            tc.tile_pool(name="ystage", bufs=8) as ysp,
        ):
            # ---- persistent SBUF tiles ----
            # chunk c holds all 8 D-row-blocks of xT for seq cols
            # [512c, 512c+512): block i at free cols [512i, 512i+512)
            xt = [pp.tile([128, DK * 512], BF, tag=f"xt{c}", name=f"xt{c}")
                  for c in range(NQC)]
            xt8 = [pp.tile([128, DK * 512], PDT, tag=f"xt8{c}", name=f"xt8{c}")
                   for c in range(NQC)]
            wq_sb = pp.tile([128, D], BF, tag="wq")
            wk_sb = pp.tile([128, D], BF, tag="wk")
            wv_sb = pp.tile([128, D], PDT, tag="wv")
            wo_sb = pp.tile([128, D], BF, tag="wo")
            bmask_sb = pp.tile([128, 1024], FP32, tag="bmask")
            mask01_sb = pp.tile([128, 1024], BF, tag="mask01")
            ident_sb = pp.tile([128, 128], BF, tag="ident")
            qT = [pp.tile([128, 512], BF, tag=f"qT{c}", name=f"qT{c}") for c in range(NQC)]
            kT = [pp.tile([128, 512], BF, tag=f"kT{c}", name=f"kT{c}") for c in range(NQC)]
            # V augmented with a 32.0 column (denominator, incl. 1/32 V-scale
            # compensation), grouped 4 k-tiles per tile
            vaug = [pp.tile([128, 4 * HL * 65], BF, tag=f"va{g}", name=f"va{g}") for g in range(NKT // 4)]
            # normalized attention output O^T, chunked by query chunk
            ot_sb = [pp.tile([128, 512], BF, tag=f"ot{c}", name=f"ot{c}") for c in range(NQC)]

            # ---- input DMAs: weights first (small, needed immediately) ----
            def load_w(w_sb, w_dram):
                nc.sync.dma_start(
                    out=w_sb[:].rearrange("p (i j) -> p i j", i=DK),
                    in_=w_dram.rearrange("(i p) j -> p i j", p=128),
                )

            xTr = xT.rearrange("(i p) s -> p i s", p=128)
            xT8r = xT8.rearrange("(i p) s -> p i s", p=128)

            def load_x_chunk(c, split=False):
                if split:
                    # two half-chunk DMAs so the first projection can start
                    # after 4 d-tiles land without paying 8x HWDGE overhead
                    for ih in range(2):
                        nc.sync.dma_start(
                            out=xt[c][:, 2048 * ih:2048 * (ih + 1)].rearrange(
                                "p (i s) -> p i s", i=DK // 2),
                            in_=xTr[:, 4 * ih:4 * ih + 4,
                                    512 * c:512 * (c + 1)],
                        )
                else:
                    nc.sync.dma_start(
                        out=xt[c][:].rearrange("p (i s) -> p i s", i=DK),
                        in_=xTr[:, :, 512 * c:512 * (c + 1)],
                    )
                nc.sync.dma_start(
                    out=xt8[c][:].rearrange("p (i s) -> p i s", i=DK),
                    in_=xT8r[:, :, 512 * c:512 * (c + 1)],
                )

            load_w(wk_sb, wk)
            load_x_chunk(0, split=True)
            load_w(wq_sb, wq)
            load_x_chunk(1)
            load_w(wv_sb, wv)
            nc.sync.dma_start(out=bmask_sb[:], in_=bmask[:])
            nc.sync.dma_start(out=mask01_sb[:], in_=mask01[:])
            nc.sync.dma_start(out=wo_sb[:], in_=wo[:])
            nc.sync.dma_start(out=ident_sb[:], in_=ident[:])
            load_x_chunk(2)
            load_x_chunk(3)
            load_x_chunk(4)

            # denominator columns of vaug (32.0 also folds away the V scale)
            for g in range(NKT // 4):
                for j in range(4 * HL):
                    nc.gpsimd.memset(vaug[g][:, j * 65 + 64:j * 65 + 65], WSCALE)

            # ---- per-chunk projections (fp8 DoubleRow), emitted interleaved
            # with attention ----
            _qk_accs = {}

            def qk_half(qc, which, half):
                w_sb, dst = (wk_sb, kT[qc]) if which == "k" else (wq_sb, qT[qc])
                if half == 0:
                    acc = sdp.tile([128, 512], FP32, tag="sd", name="mmt")
                    _qk_accs[(qc, which)] = acc
                else:
                    acc = _qk_accs.pop((qc, which))
                for i in range(4 * half, 4 * half + 4):
                    nc.tensor.matmul(
                        acc[:],
                        w_sb[:, 128 * i:128 * (i + 1)],
                        xt[qc][:, 512 * i:512 * (i + 1)],
                        start=(i == 0),
                        stop=(i == DK - 1),
                    )
                if half == 1:
                    nc.scalar.copy(dst[:], acc[:])

            def v_group(kt):
                g = kt // 4
                s0 = 128 * (kt % 4)
                acc = sdp.tile([128, 128], FP32, tag="sd", name="mmv")
                if CFG["no_fp8"]:
                    xtr = xt8[g][:].rearrange("p (i s) -> p i s", i=DK)
                    for i in range(DK):
                        nc.tensor.matmul(
                            acc[:],
                            xtr[:, i, s0:s0 + 128],
                            wv_sb[:, 128 * i:128 * (i + 1)],
                            start=(i == 0),
                            stop=(i == DK - 1),
                        )
                else:
                    xtr = xt8[g][:].rearrange("p (i s) -> p i s", i=DK)
                    for j in range(DK // 2):
                        nc.tensor.matmul(
                            acc[:],
                            xtr[:, 2 * j:2 * j + 2, s0:s0 + 128],
                            wv_sb[:, 256 * j:256 * (j + 1)].rearrange(
                                "p (two f) -> p two f", two=2),
                            start=(j == 0),
                            stop=(j == DK // 2 - 1),
                            perf_mode=DR,
                        )
                jj = kt % 4
                for h in range(HL):
                    base = (jj * HL + h) * 65
                    nc.scalar.copy(
                        vaug[g][:, base:base + 64], acc[:, 64 * h:64 * h + 64]
                    )

            def proj_units(qc):
                units = [lambda w=w, hf=hf: qk_half(qc, w, hf)
                         for w in ("k", "q") for hf in (0, 1)]
                units += [lambda kt=kt: v_group(kt) for kt in range(4 * qc, 4 * qc + 4)]
                return units

            # ---- causal attention: both heads of one query chunk, with the
            # two heads' S->exp->PV pipelines interleaved stage by stage ----
            exp_cnt = [0]

            def attention(qc, fillers, post_units):
                # PV accumulators: per head one PSUM bank holding 4 q-subtile
                # accumulators of 65 cols (64 O-cols + denominator) each
                pvacc = {hh: pvp.tile([128, 4 * 65], FP32, tag="pv",
                                      name=f"pvacc{hh}") for hh in range(HL)}
                pv_started = {hh: False for hh in range(HL)}
                # normalized O for the chunk: [128 q, (4 qsub x 128 d)]
                o_norm = onp.tile([128, 4 * 128], BF, tag="on", name="onorm")
                st_tiles = {}
                e_tiles = {}

                def kslice(h, kt):
                    return kT[kt // 4][
                        64 * h:64 * h + 64, 128 * (kt % 4):128 * (kt % 4) + 128
                    ]

                def vslice(h, kt):
                    base = ((kt % 4) * HL + h) * 65
                    return vaug[kt // 4][:, base:base + 65]

                def pv_block(h, e_ap, ecol, qsub, kt, stop):
                    # pvacc[h][:, 65*qsub : 65*qsub+65] += e_block^T @ vaug
                    start = not pv_started[h]
                    pv_started[h] = True
                    nc.tensor.matmul(
                        pvacc[h][:, 65 * qsub:65 * qsub + 65],
                        e_ap[:, ecol:ecol + 128].bitcast(BF),
                        vslice(h, kt),
                        start=start,
                        stop=stop,
                        skip_group_check=True,
                    )

                # one k-tile per stage; stage key: ("d", j) diag / ("f", t)
                def dwidth(j):
                    return 512 - 128 * j

                def s_stage(h, kind, j):
                    if kind == "d":
                        w = dwidth(j)
                        st = stp.tile([128, w], FP32, tag="st", name="std")
                        nc.tensor.matmul(
                            st[:],
                            kslice(h, 4 * qc + j),
                            qT[qc][64 * h:64 * h + 64, 512 - w:512],
                            start=True, stop=True,
                        )
                    else:
                        st = stp.tile([128, 512], FP32, tag="st", name="stf")
                        nc.tensor.matmul(
                            st[:],
                            kslice(h, j),
                            qT[qc][64 * h:64 * h + 64, :],
                            start=True, stop=True,
                        )
                    st_tiles[(h, kind, j)] = st

                def e_stage(h, kind, j):
                    st = st_tiles.pop((h, kind, j))
                    if kind == "d":
                        w = dwidth(j)
                        e = ep.tile([128, w], BF, tag="e", name="etd")
                        if CFG["no_fastexp"]:
                            nc.scalar.activation(e[:], st[:], EXP,
                                                 scale=SSCALE)
                            nc.vector.tensor_mul(
                                e[:], e[:], mask01_sb[:, 512:512 + w])
                        else:
                            # fused fast-exp + causal mask on DVE
                            nc.vector.scalar_tensor_tensor(
                                e[:].bitcast(U16), st[:], FE_A,
                                bmask_sb[:, 512:512 + w], MULT, ADD)
                    else:
                        e = ep.tile([128, 512], BF, tag="e", name="etf")
                        if (not CFG["no_fastexp"]) and exp_cnt[0] % CFG["act_den"] >= CFG["act_num"]:
                            nc.vector.tensor_scalar(
                                e[:].bitcast(U16), st[:], FE_A, FE_B, MULT, ADD)
                        else:
                            nc.scalar.activation(e[:], st[:], EXP, scale=SSCALE)
                        exp_cnt[0] += 1
                    e_tiles[(h, kind, j)] = e

                def v_stage(h, kind, j):
                    e = e_tiles.pop((h, kind, j))
                    if kind == "d":
                        kt = 4 * qc + j
                        for i in range(j, 4):
                            pv_block(h, e, 128 * (i - j), i, kt,
                                     stop=(qc == 0 and i == j))
                    else:
                        for i in range(4):
                            pv_block(h, e, 128 * i, i, j,
                                     stop=(j == 4 * qc - 1))

                # normalize head hh: per-partition scalar = 1/denominator
                # (denominator column already includes the 32x V-scale)
                def div(hh):
                    pv = pvacc[hh][:].rearrange("p (i c) -> p i c", c=65)
                    rd = sp.tile([128, 4], FP32, tag="rd", name="rdt")
                    nc.vector.reciprocal(rd[:], pv[:, :, 64])
                    onv = o_norm[:].rearrange("p (i d) -> p i d", d=128)
                    nc.vector.tensor_mul(
                        onv[:, :, 64 * hh:64 * hh + 64],
                        pv[:, :, 0:64],
                        rd[:].broadcast_to([128, 4, 64]),
                    )

                # software-pipelined emission: PE alternates S and PV so the
                # exp latency is hidden; heads interleaved stage by stage
                per_head = [("d", j) for j in range(4)] + [
                    ("f", t) for t in range(4 * qc)]
                stages = []
                for st_ in per_head:
                    stages.append((0, *st_))
                    stages.append((1, *st_))

                last_stage = {hh: max(i for i, s_ in enumerate(stages)
                                      if s_[0] == hh) for hh in range(HL)}
                nstages = len(stages)
                LAG = min(CFG["lag"], nstages)  # slots between exp emit and PV emit

                def emit_pv_slot(jj):
                    vh, vk, vi = stages[jj]
                    v_stage(vh, vk, vi)
                    if jj == last_stage[vh]:
                        div(vh)

                for i, (hh, kind, idx) in enumerate(stages):
                    s_stage(hh, kind, idx)
                    if i >= LAG:
                        emit_pv_slot(i - LAG)
                    e_stage(hh, kind, idx)
                    # spread remaining fillers evenly over remaining stages
                    rem = nstages - i
                    if fillers and len(fillers) >= rem:
                        for _ in range(-(-len(fillers) // rem)):
                            if fillers:
                                fillers.pop(0)()
                    elif fillers and (i * len(fillers)) // nstages != ((i + 1) * len(fillers)) // nstages:
                        fillers.pop(0)()
                for jj in range(max(0, nstages - LAG), nstages):
                    emit_pv_slot(jj)

                # transpose O [q, d] -> O^T [d, q] for the output projection;
                # deferred into the next chunk's filler stream so the serial
                # normalize chain does not stall the in-order PE queue
                otT_cell = []

                def transpose_one(i):
                    if not otT_cell:
                        otT_cell.append(
                            sdp.tile([128, 512], BF, tag="sd", name="otT"))
                    otT = otT_cell[0]
                    onv = o_norm[:].rearrange("p (i d) -> p i d", d=128)
                    nc.tensor.transpose(
                        otT[:, 128 * i:128 * (i + 1)], onv[:, i, :],
                        ident_sb[:]
                    )
                    nc.vector.tensor_copy(ot_sb[qc][:, 128 * i:128 * (i + 1)],
                                           otT[:, 128 * i:128 * (i + 1)])
                for i in range(4):
                    post_units.append(lambda i=i: transpose_one(i))

            y_cnt = [0]

            def wo_unit(qc, t, n):
                qt = 4 * qc + t
                acc = sdp.tile([128, 512], FP32, tag="sd", name="yacc")
                nc.tensor.matmul(
                    acc[:],
                    ot_sb[qc][:, 128 * t:128 * (t + 1)],
                    wo_sb[:, 512 * n:512 * (n + 1)],
                    start=True,
                    stop=True,
                )
                ys = ysp.tile([128, 512], BF, tag="ys", name="yst")
                if y_cnt[0] % CFG["yact_den"] < CFG["yact_num"]:
                    nc.scalar.copy(ys[:], acc[:])
                else:
                    nc.vector.tensor_copy(ys[:], acc[:])
                y_cnt[0] += 1
                nc.sync.dma_start(
                    out=y[128 * qt:128 * (qt + 1), 512 * n:512 * (n + 1)],
                    in_=ys[:],
                )

            def wo_units(qc):
                return [lambda t=t, n=n: wo_unit(qc, t, n)
                        for t in range(4) for n in range(2)]

            for _rep in range(repeat):
              # projections run 2 chunks ahead of attention so the PSUM->SBUF
              # copies of qT/kT/vaug are never on the critical path
              PA = CFG["proj_ahead"]
              for c0 in range(PA):
                  for u in proj_units(c0):
                      u()
              pending_post = []
              for qc in range(NQC):
                  if qc + 5 < NQC:
                      load_x_chunk(qc + 5)   # prefetch 5 chunks ahead
                  fillers = list(pending_post)
                  pending_post = []
                  if qc + PA < NQC:
                      fillers += proj_units(qc + PA)
                  if qc >= 1:
                      fillers += wo_units(qc - 1)
                  attention(qc, fillers, pending_post)
                  for u in fillers:   # drain leftovers
                      u()
                  fillers.clear()
              last_wo = wo_units(NQC - 1)
              for i in range(4):
                  pending_post[i]()
                  last_wo[2 * i]()
                  last_wo[2 * i + 1]()

    nc.compile()
    return nc


_program = None


def _get_program():
    global _program
    if _program is None:
        _program = _build_program()
    return _program


def _make_bmask():
    t = np.arange(1024)[None, :]
    k = np.arange(128)[:, None]
    # masked entries get +8192 instead of +FE_B: the uint16 stays positive
    # (no wrap/saturate risk) and the bf16 bit pattern 8192+eps ~ 2^-95 ~ 0
    return np.where(k <= t - 512, np.float32(FE_B), np.float32(8192.0))


def _make_ident():
    return np.eye(128, dtype=BF16)


def _make_in_maps(x, Wq, Wk, Wv, Wo):
    FP8NP = mybir.dt.np(BF if CFG["no_fp8"] else FP8)
    xTf = np.ascontiguousarray(x[0].T)
    xT = xTf.astype(BF16)
    xT8 = xTf.astype(FP8NP)
    bmask = _make_bmask()
    t = np.arange(1024)[None, :]
    k = np.arange(128)[:, None]
    mask01 = (k <= t - 512).astype(BF16)
    ident = _make_ident()
    in_maps = []
    for c in range(NCORES):
        hs = slice(128 * c, 128 * (c + 1))
        in_maps.append({
            "xT": xT,
            "xT8": xT8,
            "wq": np.ascontiguousarray(Wq[:, hs]).astype(BF16),
            "wk": np.ascontiguousarray(Wk[:, hs]).astype(BF16),
            "wv": np.ascontiguousarray(Wv[:, hs] * WSCALE).astype(FP8NP),
            "wo": np.ascontiguousarray(Wo[hs, :]).astype(BF16),
            "bmask": bmask,
            "mask01": mask01,
            "ident": ident,
        })
    return in_maps


def kernel(x, Wq, Wk, Wv, Wo):
    x = np.asarray(x, dtype=np.float32)
    Wq, Wk, Wv, Wo = (np.asarray(w, dtype=np.float32) for w in (Wq, Wk, Wv, Wo))
    nc = _get_program()
    in_maps = _make_in_maps(x, Wq, Wk, Wv, Wo)

    res = run_bass_kernel_spmd(nc, in_maps, core_ids=list(range(NCORES)))
    out = np.zeros((S, D), np.float32)
    for c in range(NCORES):
        out += np.asarray(res.results[c]["y"], dtype=np.float32)
    return out.reshape(1, S, D)
